# revision 35
# baseline (speedup 1.0000x reference)
"""Trainium2 Bass kernel for the CIntegration embedding-lookup module.

reference semantics (all fp32):
    ct    = concat(one_hot(rgap, 32), one_hot(sgap, 32), one_hot(pcount, 64))  # [B,S,128]
    Cct   = W.T[rgap] + W.T[32+sgap] + W.T[64+pcount]                          # [B,S,128]
    theta = vt * Cct
    out   = concat(theta, ct)                                                  # [B,S,256]

Strategy (8 NeuronCores, data-parallel over the batch dim, W replicated):
  The correctness gate is rel-err < 2e-2 of the output scale, which admits
  16-bit I/O end to end (and fp8 for the exact 0/1 one-hot block).  DMA
  bandwidth is the binding constraint (memory regime): the HBM floor is
  vt(bf16 8.4MB) + idx(0.2MB) + theta(bf16 8.4MB) + ct(fp8 4.2MB) ~ 21MB
  per core.

  Transposed on-chip layout: SBUF partition p = embedding/bin index, free
  dim = token (natural order).  The combined bin-index stream is loaded
  ONCE as a compact [4, T] tensor (rows 0-2: group indices; row 3 is a
  ones/-iota row for an alternate path) and broadcast on-chip per
  512-token slice with a K=3 PE matmul against a 0/1 group-selection
  matrix (partition p of the output gets the idx stream of the bin-block
  it belongs to).  This replaces the baseline's host-replicated [128, T]
  bf16 b3rep stream (8.4MB of pure HBM overhead per core).  Per 2048-
  token chunk:
    - b3ps[p, q512] = sel.T @ idx3 (PE, PSUM f32, exact for ints < 128)
    - b3sb = copy(b3ps) (ACT, PSUM f32 -> SBUF bf16, exact)
    - ctT[bin, t] = (b3sb == iota): DVE is_equal in the fast all-SBUF
      16-bit 4x mode, written straight into the output layout; the same
      bytes are the moving operand of the W-gather matmul.
    - CctT = Wt.T @ ctT (PE, stationary Wt bf16; alternates with sel).
    - thetaT = vtT * CctT (DVE multiply, PSUM source).
    - stores: BOTH outputs via SWDGE on the Pool queue — theta as a plain
      bf16 DMA, ct as a cast-DMA (bf16 SBUF -> fp8 HBM, which also removes
      the ACT fp8 convert).  A store trigger is an instruction in the
      issuing engine's strict-FIFO queue; on a compute engine (ACT) it
      waits for its producer at the queue head and blocks the next chunk's
      PSUM->SBUF copies behind it (measured +4.3us).  The Pool queue runs
      no compute, so its store triggers block nothing.
  Measured on HW the kernel is purely DMA-bound (a transfers-only variant
  times the same); 512KB chunk-granular DMAs (vt loads on SP-HWDGE, both
  stores on SWDGE) measured faster than every coarser pairing,
  lagged-store, ring-swap, and alternate-compare variant tried (see the
  session sweeps).  The bench For_i loop uses staggered_reset (no
  all-engine barrier per iteration); looped outputs validated
  bit-identical to the single-shot run.
"""

import sys

import numpy as np

try:  # concourse is on sys.path via sitecustomize in the runtime image;
    import concourse  # noqa: F401  # fall back to known locations otherwise
except ImportError:  # pragma: no cover
    for _p in ("/opt/trn_rl_repo", "/root/.axon_site/_ro/trn_rl_repo"):
        if _p not in sys.path:
            sys.path.insert(0, _p)

B, S, EMB = 256, 1024, 128
NUM_RGAP, NUM_SGAP, NUM_PCOUNT = 32, 32, 64
NTOTAL = NUM_RGAP + NUM_SGAP + NUM_PCOUNT  # 128
NCORES = 8
ROWS_PER_CORE = B // NCORES                # 32
T_CORE = ROWS_PER_CORE * S                 # 32768 tokens per core
CHUNK = 2048                               # tokens per compute chunk
NCHUNK = T_CORE // CHUNK                   # 16
PAIR = 2                                   # chunks per DMA block
NPAIR = NCHUNK // PAIR                     # 8
QMM = CHUNK // 512                         # matmuls per chunk (512-col)

_compiled = {}

# deployment build config (kernel() and test harness use this).
# th_eng="gpsimd": store triggers live on the Pool queue, which runs no
# compute — on the ACT queue they wait for their mult at the queue head
# and block the next chunk's PSUM->SBUF copies (measured +4.3us).
BEST_KW = dict(pair=1, bufs=6, cast_store=True, staggered=True, th_eng="gpsimd")


def _build_program(
    loop_n=None,
    pair=PAIR,
    bufs=3,
    cast_store=True,
    cc_cols=1024,
    ct_eng="sync",
    th_eng="scalar",
    vt_eng="sync",
    th_half=False,
    first_split=False,
    act_onehot=False,
    staggered=False,
    chunk=CHUNK,
    body_rep=1,
    dma_only=False,
    cc_sbuf=False,
    st_lag=0,
    ld_pair=None,
    st_pair=None,
    vbufs=None,
    tt_iseq=False,
    gp_iseq=False,
    sp_dma=False,
    ct_first=False,
):
    import concourse.bacc as bacc
    import concourse.mybir as mybir
    from concourse import tile

    f32 = mybir.dt.float32
    bf16 = mybir.dt.bfloat16
    fp8 = mybir.dt.float8e4
    Alu = mybir.AluOpType

    CHUNK_ = chunk
    QMM_ = CHUNK_ // 512
    cc_cols = min(cc_cols, CHUNK_)
    NCHUNK_ = T_CORE // CHUNK_
    ld_pair = ld_pair or pair
    st_pair = st_pair or pair
    ld_blk = ld_pair * CHUNK_
    st_blk = st_pair * CHUNK_
    nld = NCHUNK_ // ld_pair
    nst = NCHUNK_ // st_pair

    nc = bacc.Bacc(None)

    vt_in = nc.declare_dram_parameter("vt", [nld, 128, ld_blk], bf16, isOutput=False)
    # row 3 of idx3/sel: ones / -iota (only consumed by the act_onehot path)
    idx_in = nc.declare_dram_parameter("idx3", [4, T_CORE], bf16, isOutput=False)
    wt_in = nc.declare_dram_parameter("wt", [128, 128], bf16, isOutput=False)
    sel_in = nc.declare_dram_parameter("sel", [4, 128], bf16, isOutput=False)
    iota_in = nc.declare_dram_parameter("iota_col", [128, 1], f32, isOutput=False)
    th_ext = nc.declare_dram_parameter(
        "th_out", [nst, 128, st_blk], bf16, isOutput=True
    )
    ct_ext = nc.declare_dram_parameter(
        "ct_out", [nst, 128, st_blk], fp8, isOutput=True
    )

    with tile.TileContext(nc) as tc:
        with (
            tc.tile_pool(name="consts", bufs=1) as consts,
            tc.tile_pool(name="vt", bufs=vbufs or bufs) as vtp,
            tc.tile_pool(name="b3", bufs=bufs) as b3p,
            tc.tile_pool(name="ctb", bufs=bufs) as ctp,
            tc.tile_pool(name="th", bufs=bufs) as thp,
            tc.tile_pool(name="ct8", bufs=bufs) as ct8p,
            tc.tile_pool(name="ccs", bufs=bufs) as ccsp,
            tc.tile_pool(name="ps_bc", bufs=4, space="PSUM") as psbc,
            tc.tile_pool(name="ps_cc", bufs=2, space="PSUM") as pscc,
        ):
            idx3 = consts.tile([4, T_CORE], bf16, tag="idx3")
            nc.gpsimd.dma_start(out=idx3[:, :], in_=idx_in[:, :])
            wt = consts.tile([128, 128], bf16, tag="wt")
            sel = consts.tile([4, 128], bf16, tag="sel")
            iota = consts.tile([128, 1], f32, tag="iota")
            nc.gpsimd.dma_start(out=wt[:, :], in_=wt_in[:, :])
            nc.gpsimd.dma_start(out=sel[:, :], in_=sel_in[:, :])
            nc.scalar.dma_start(out=iota[:, :], in_=iota_in[:, :])
            iota_rep = None
            if tt_iseq:
                # per-partition constant replicated along the free dim: lets
                # the one-hot compare run as tensor_tensor (2x_1P, single
                # SBUF port) instead of tensor_scalar (4x_2P, which locks
                # GPSIMD out of the shared port pair while SWDGE stores
                # need to generate descriptors)
                iota_rep = consts.tile([128, CHUNK_], bf16, tag="iota_rep")
                nc.vector.tensor_scalar(
                    out=iota_rep[:, :],
                    in0=iota[:, 0:1].broadcast_to((128, CHUNK_)),
                    scalar1=0.0,
                    scalar2=None,
                    op0=Alu.add,
                )
            nrows = 4 if act_onehot else 3  # K of the broadcast matmul

            def emit_store(sp_i, th, ct_bf):
                if ct_first and cast_store:
                    # ct depends on iseq, th on the (later) mult: emit ct
                    # first so it is not queue-blocked behind th's wait
                    nc.gpsimd.dma_start(out=ct_ext[sp_i], in_=ct_bf[:, :])
                getattr(nc, th_eng).dma_start(
                    out=th_ext[sp_i], in_=th[:, :], single_packet=sp_dma
                )
                if cast_store:
                    if not ct_first:
                        # SWDGE cast-DMA: bf16 SBUF -> fp8 HBM
                        nc.gpsimd.dma_start(out=ct_ext[sp_i], in_=ct_bf[:, :])
                else:
                    ct8 = ct8p.tile([128, st_blk], fp8, tag="ct8")
                    nc.scalar.copy(out=ct8[:, :], in_=ct_bf[:, :])
                    getattr(nc, ct_eng).dma_start(out=ct_ext[sp_i], in_=ct8[:, :])

            def body():
                if dma_only:
                    # same HBM transfers, no compute: isolates DMA capability
                    for c in range(NCHUNK_):
                        lp, lo = divmod(c, ld_pair)
                        if lo == 0:
                            vt_sb = vtp.tile([128, ld_blk], bf16, tag="vt")
                            getattr(nc, vt_eng).dma_start(
                                out=vt_sb[:, :], in_=vt_in[lp]
                            )
                        sp_i, so = divmod(c, st_pair)
                        if so == st_pair - 1:
                            getattr(nc, th_eng).dma_start(
                                out=th_ext[sp_i],
                                in_=vt_sb[:, : st_blk] if st_blk <= ld_blk else vt_sb[:, :],
                            )
                            nc.gpsimd.dma_start(
                                out=ct_ext[sp_i], in_=vt_sb[:, : st_blk]
                            )
                    return
                pending = []
                vt_sb = ct_bf = th = None
                for c in range(NCHUNK_):
                    lp, lo = divmod(c, ld_pair)
                    if lo == 0:
                        vt_sb = vtp.tile([128, ld_blk], bf16, tag="vt")
                        if vt_eng == "alt":
                            # alternate loads across the two HWDGE rings
                            veng = nc.sync if lp % 2 == 0 else nc.scalar
                        else:
                            veng = getattr(nc, vt_eng)
                        veng.dma_start(
                            out=vt_sb[:, :], in_=vt_in[lp], single_packet=sp_dma
                        )
                    sp_i, so = divmod(c, st_pair)
                    if so == 0:
                        ct_bf = ctp.tile([128, st_blk], bf16, tag="ctb")
                        th = thp.tile([128, st_blk], bf16, tag="th")
                    cbase = c * CHUNK_   # into idx3
                    lbase = lo * CHUNK_  # into vt_sb
                    sbase = so * CHUNK_  # into ct_bf / th
                    # broadcast compact idx to all 128 partitions (PE);
                    # act_onehot also subtracts iota via the 4th sel row
                    b3sb = b3p.tile([128, CHUNK_], bf16, tag="b3sb")
                    for q in range(QMM_):
                        b3ps = psbc.tile([128, 512], f32, tag="b3ps")
                        nc.tensor.matmul(
                            b3ps[:, :],
                            sel[:nrows, :],
                            idx3[:nrows, cbase + q * 512 : cbase + (q + 1) * 512],
                            start=True,
                            stop=True,
                        )
                        if act_onehot:
                            nc.scalar.activation(
                                out=b3sb[:, q * 512 : (q + 1) * 512],
                                in_=b3ps[:, :],
                                func=mybir.ActivationFunctionType.Abs,
                            )
                        else:
                            nc.scalar.copy(
                                out=b3sb[:, q * 512 : (q + 1) * 512], in_=b3ps[:, :]
                            )
                    if act_onehot:
                        # one-hot: relu(1 - |idx - p|) on ACT (ints -> exact)
                        nc.scalar.activation(
                            out=ct_bf[:, sbase : sbase + CHUNK_],
                            in_=b3sb[:, :],
                            func=mybir.ActivationFunctionType.Relu,
                            bias=1.0,
                            scale=-1.0,
                        )
                    elif tt_iseq:
                        nc.vector.tensor_tensor(
                            out=ct_bf[:, sbase : sbase + CHUNK_],
                            in0=b3sb[:, :],
                            in1=iota_rep[:, :],
                            op=Alu.is_equal,
                        )
                    elif gp_iseq:
                        nc.gpsimd.tensor_scalar(
                            out=ct_bf[:, sbase : sbase + CHUNK_],
                            in0=b3sb[:, :],
                            scalar1=iota[:, :],
                            scalar2=None,
                            op0=Alu.is_equal,
                        )
                    else:
                        # one-hot: DVE is_equal in fast all-SBUF 16-bit mode
                        nc.vector.tensor_scalar(
                            out=ct_bf[:, sbase : sbase + CHUNK_],
                            in0=b3sb[:, :],
                            scalar1=iota[:, :],
                            scalar2=None,
                            op0=Alu.is_equal,
                        )
                    # W-gather + theta multiply per cc_cols columns
                    for k in range(CHUNK_ // cc_cols):
                        cc = pscc.tile([128, cc_cols], f32, tag="cc")
                        for q2 in range(cc_cols // 512):
                            off = sbase + k * cc_cols + q2 * 512
                            nc.tensor.matmul(
                                cc[:, q2 * 512 : (q2 + 1) * 512],
                                wt[:, :],
                                ct_bf[:, off : off + 512],
                                start=True,
                                stop=True,
                            )
                        koff = k * cc_cols
                        if cc_sbuf:
                            ccs = ccsp.tile([128, cc_cols], bf16, tag="ccs")
                            nc.scalar.copy(out=ccs[:, :], in_=cc[:, :])
                            mul_in1 = ccs[:, :]
                        else:
                            mul_in1 = cc[:, :]
                        nc.vector.tensor_tensor(
                            out=th[:, sbase + koff : sbase + koff + cc_cols],
                            in0=vt_sb[:, lbase + koff : lbase + koff + cc_cols],
                            in1=mul_in1,
                            op=Alu.mult,
                        )
                    if so == st_pair - 1:
                        pending.append((sp_i, th, ct_bf))
                        while len(pending) > st_lag:
                            emit_store(*pending.pop(0))
                while pending:
                    emit_store(*pending.pop(0))

            if loop_n is None:
                for _ in range(body_rep):
                    body()
            else:
                with tc.For_i(0, loop_n, staggered_reset=staggered):
                    for _ in range(body_rep):
                        body()

    nc.compile()
    return nc


def _get_compiled(loop_n=None, **kw):
    key = (loop_n, tuple(sorted(kw.items())))
    if key not in _compiled:
        _compiled[key] = _build_program(loop_n, **kw)
    return _compiled[key]


_GROUP_ROWS = np.repeat(np.arange(3), [NUM_RGAP, NUM_SGAP, NUM_PCOUNT])  # [128]


def _host_prep(vt, rgap, sgap, pcount, W, ld_pair=1):
    import concourse.mybir as mybir

    bf16 = mybir.dt.np(mybir.dt.bfloat16)

    vt = np.asarray(vt)
    W = np.asarray(W, dtype=np.float32)
    rgap = np.asarray(rgap)
    sgap = np.asarray(sgap)
    pcount = np.asarray(pcount)

    wt = np.ascontiguousarray(W.T.astype(bf16))  # [bin, emb]
    # rows 0-2: 0/1 group-selection; row 3: -iota (for the act_onehot path)
    sel = np.concatenate(
        [
            (_GROUP_ROWS[None, :] == np.arange(3)[:, None]).astype(np.float32),
            -np.arange(128, dtype=np.float32)[None, :],
        ]
    ).astype(bf16)  # [4, 128]
    iota_col = np.arange(128, dtype=np.float32).reshape(128, 1)

    # combined bin indices (int values < 128, exact in bf16); row 3: ones
    idx = np.concatenate(
        [
            np.stack([rgap, NUM_RGAP + sgap, NUM_RGAP + NUM_SGAP + pcount]),
            np.ones((1, B, S), dtype=np.int64),
        ]
    ).astype(np.float32).astype(bf16)  # [4, B, S]

    ld_blk = ld_pair * CHUNK
    in_maps = []
    for core in range(NCORES):
        r0 = core * ROWS_PER_CORE
        # vtT per DMA block: [emb, token] with tokens in natural order
        vt_c = np.ascontiguousarray(
            vt[r0 : r0 + ROWS_PER_CORE]
            .reshape(T_CORE // ld_blk, ld_blk, EMB)
            .transpose(0, 2, 1)
            .astype(bf16)
        )
        idx_c = np.ascontiguousarray(
            idx[:, r0 : r0 + ROWS_PER_CORE, :].reshape(4, T_CORE)
        )
        in_maps.append(
            {"vt": vt_c, "idx3": idx_c, "wt": wt, "sel": sel, "iota_col": iota_col}
        )
    return in_maps


def _run(in_maps, trace=False, loop_n=None, **kw):
    from concourse.bass_utils import run_bass_kernel_spmd

    nc = _get_compiled(loop_n, **kw)
    # transient device wedges (NRT_EXEC_UNIT_UNRECOVERABLE) recover on rerun
    last_err = None
    for _ in range(3):
        try:
            return run_bass_kernel_spmd(nc, in_maps, list(range(NCORES)), trace=trace)
        except Exception as e:  # noqa: BLE001
            if "UNRECOVERABLE" not in str(e) and "UNAVAILABLE" not in str(e):
                raise
            last_err = e
    raise last_err


def _unshard(res):
    outs = []
    for core in range(NCORES):
        r = res.results[core]
        th = r["th_out"].astype(np.float32)  # [nst, 128, st_blk]
        ct = r["ct_out"].astype(np.float32)
        o = np.stack([th, ct], axis=2)  # [nst, 128, 2, st_blk]
        # [b, p, k, t] -> token b*st_blk + t, feature k*128 + p
        o = o.transpose(0, 3, 2, 1).reshape(ROWS_PER_CORE, S, 2 * EMB)
        outs.append(o)
    return np.ascontiguousarray(np.concatenate(outs, axis=0))


def kernel(vt, rgap, sgap, pcount, W):
    in_maps = _host_prep(vt, rgap, sgap, pcount, W, ld_pair=BEST_KW.get("ld_pair", 1))
    res = _run(in_maps, **BEST_KW)
    return _unshard(res)


if __name__ == "__main__":
    rng = np.random.default_rng(0)
    vt = rng.standard_normal((B, S, EMB), dtype=np.float32)
    rgap = rng.integers(0, NUM_RGAP, (B, S))
    sgap = rng.integers(0, NUM_SGAP, (B, S))
    pcount = rng.integers(0, NUM_PCOUNT, (B, S))
    W = (rng.standard_normal((EMB, NTOTAL)) * 0.05).astype(np.float32)
    out = kernel(vt, rgap, sgap, pcount, W)
    print(out.shape, out.dtype)
